# revision 1
# baseline (speedup 1.0000x reference)
"""v3: fp32r conv1 (15 matmuls/tile) + bf16 hi/lo conv2 (col-tiled pairs).

conv1 (fp32r, FP22 internally, ~2^-12):
  - kd in {0,1} packed into K=128 via xpair tiles (low=slice s, high=s+1): 9 MMs
  - kd=2: kh in {0,1} packed via pairC tiles (low=slice s+2, high=s+2
    shifted one row): 3 MMs K=128; kh=2 leftover: 3 MMs K=64.
conv2 (bf16): inputs are integer-valued (exact in bf16); weights split
  hi/lo into K=128 (rows 0-63 hi, 64-127 lo), activations duplicated in
  both halves of qdup tiles: 27 MMs K=128 per tile, run as column-tiled
  concurrent pairs (tile_position=(0,64)) -> 2 tiles per PE pass.

BFP quant: ACT (scale+bias) -> DVE add-magic/clamp (exact fp32 RNE).
"""

import numpy as np
import ml_dtypes
import concourse.mybir as mybir
from concourse import bacc
from concourse.tile import TileContext
from concourse.bass_utils import run_bass_kernel_spmd

BF16 = mybir.dt.bfloat16
F32R = mybir.dt.float32r
F32 = mybir.dt.float32

MANTISA_BIT = 8.0
MAGIC = 12582912.0

N, C, T, H, W = 8, 64, 16, 56, 56
TP, HP, WP = T + 2, H + 2, W + 2
PLANE = HP * WP
SLICE = H * W
ROWS = 7
NT = ROWS * W  # 392
NTILES = H // ROWS  # 8
NPAIR = NTILES // 2

_COMPILED = None


def _border_memset(nc, tile, cast=None):
    v = tile[:]
    if cast is not None:
        v = v.bitcast(cast)
    v = v.rearrange("p (h w) -> p h w", w=WP)
    nc.gpsimd.memset(v[:, 0, :], 0.0)
    nc.gpsimd.memset(v[:, HP - 1, :], 0.0)
    nc.gpsimd.memset(v[:, 1 : HP - 1, 0], 0.0)
    nc.gpsimd.memset(v[:, 1 : HP - 1, WP - 1], 0.0)


def _build():
    nc = bacc.Bacc()
    xpad_d = nc.declare_dram_parameter("xpad", [C, TP, PLANE], F32R, isOutput=False)
    x32_d = nc.declare_dram_parameter("x32", [128, T, SLICE // 2], F32, isOutput=False)
    w1p9_d = nc.declare_dram_parameter("w1p9", [128, 9 * 64], F32R, isOutput=False)
    w1c3_d = nc.declare_dram_parameter("w1c3", [128, 3 * 64], F32R, isOutput=False)
    w1k3_d = nc.declare_dram_parameter("w1k3", [64, 3 * 64], F32R, isOutput=False)
    w2hl_d = nc.declare_dram_parameter("w2hl", [128, 27 * 64], BF16, isOutput=False)
    coeff_d = nc.declare_dram_parameter("coeff", [128, 8], F32, isOutput=False)
    out_d = nc.declare_dram_parameter("out", [C, T * SLICE], F32, isOutput=True)

    with TileContext(nc) as tc:
        with (
            tc.tile_pool(name="big", bufs=1) as bigpool,
            tc.tile_pool(name="xp", bufs=3) as xpool,
            tc.tile_pool(name="xc", bufs=3) as xcpool,
            tc.tile_pool(name="qd", bufs=5) as qdpool,
            tc.tile_pool(name="x3", bufs=2) as x3pool,
            tc.tile_pool(name="small", bufs=4) as spool,
            tc.tile_pool(name="ps1", bufs=5, space="PSUM") as ps1pool,
            tc.tile_pool(name="ps2", bufs=3, space="PSUM") as ps2pool,
        ):
            w1p9 = bigpool.tile([128, 9 * 64], F32R, tag="w1p9")
            nc.sync.dma_start(out=w1p9[:], in_=w1p9_d[:])
            w1c3 = bigpool.tile([128, 3 * 64], F32R, tag="w1c3")
            nc.sync.dma_start(out=w1c3[:], in_=w1c3_d[:])
            w1k3 = bigpool.tile([64, 3 * 64], F32R, tag="w1k3")
            nc.sync.dma_start(out=w1k3[:], in_=w1k3_d[:])
            w2hl = bigpool.tile([128, 27 * 64], BF16, tag="w2hl")
            nc.sync.dma_start(out=w2hl[:], in_=w2hl_d[:])
            coeff = bigpool.tile([128, 8], F32, tag="coeff")
            nc.sync.dma_start(out=coeff[:], in_=coeff_d[:])
            zerot = bigpool.tile([128, PLANE], BF16, tag="zerot")
            nc.gpsimd.memset(zerot[:], 0.0)

            s1 = coeff[0:64, 0:1]
            b1s1 = coeff[0:64, 1:2]
            s2 = coeff[:, 2:3]
            b2s2 = coeff[:, 3:4]
            inv_s2 = coeff[:, 4:5]
            negM = coeff[:, 5:6]

            xpair = {}
            pairC = {}

            def load_x(s):
                # xpair[s]: low = x_pad[s], high = x_pad[s+1]
                tp_ = xpool.tile([128, PLANE], F32R, tag="xpair")
                nc.sync.dma_start(out=tp_[0:64, :], in_=xpad_d[:, s, :])
                nc.sync.dma_start(out=tp_[64:128, :], in_=xpad_d[:, s + 1, :])
                xpair[s] = tp_
                # pairC[s]: low = x_pad[s+2], high = x_pad[s+2] shifted +1 row
                tc_ = xcpool.tile([128, PLANE], F32R, tag="pairC")
                nc.sync.dma_start(out=tc_[0:64, :], in_=xpad_d[:, s + 2, :])
                nc.sync.dma_start(
                    out=tc_[64:128, 0 : PLANE - WP], in_=xpad_d[:, s + 2, WP:PLANE]
                )
                pairC[s] = tc_

            for s in range(2):
                load_x(s)

            qdup = {}

            def alloc_qdup(s):
                t_ = qdpool.tile([128, PLANE], BF16, tag="qdup")
                if s == 0:
                    nc.gpsimd.memset(t_[:], 0.0)
                else:
                    _border_memset(nc, t_)
                qdup[s] = t_

            alloc_qdup(0)

            def pview(ap):
                return ap.rearrange("p (h w) -> p h w", w=WP)

            for t in range(T + 1):
                if t < T:
                    if t + 2 <= T - 1:
                        load_x(t + 2)
                    if t + 1 <= 16:
                        alloc_qdup(t + 1)
                    xp_v = pview(xpair[t][:])
                    xc_v = pview(pairC[t][:])
                    xc64_v = pview(pairC[t][0:64, :])
                    lo_v = pview(qdup[t + 1][:])
                    for j in range(NTILES):
                        r0 = j * ROWS
                        ps = ps1pool.tile([64, NT], F32, tag="ps1")
                        k = 0
                        for kh in range(3):
                            for kw in range(3):
                                nc.tensor.matmul(
                                    ps[:],
                                    w1p9[:, 64 * (3 * kh + kw) : 64 * (3 * kh + kw) + 64],
                                    xp_v[:, r0 + kh : r0 + kh + ROWS, kw : kw + W],
                                    start=(k == 0), stop=False,
                                )
                                k += 1
                        for kw in range(3):
                            nc.tensor.matmul(
                                ps[:], w1c3[:, 64 * kw : 64 * kw + 64],
                                xc_v[:, r0 : r0 + ROWS, kw : kw + W],
                                start=False, stop=False,
                            )
                        for kw in range(3):
                            nc.tensor.matmul(
                                ps[:], w1k3[:, 64 * kw : 64 * kw + 64],
                                xc64_v[:, r0 + 2 : r0 + 2 + ROWS, kw : kw + W],
                                start=False, stop=(kw == 2),
                            )
                        r_sb = spool.tile([64, NT], F32, tag="r1")
                        nc.scalar.activation(
                            r_sb[:], ps[:], mybir.ActivationFunctionType.Relu,
                            bias=b1s1, scale=s1,
                        )
                        m_sb = spool.tile([64, NT], F32, tag="m1")
                        nc.vector.tensor_scalar(
                            out=m_sb[:], in0=r_sb[:],
                            scalar1=MAGIC, scalar2=MAGIC + 127.0,
                            op0=mybir.AluOpType.add, op1=mybir.AluOpType.min,
                        )
                        q_sb = spool.tile([64, NT], BF16, tag="q1")
                        nc.vector.tensor_scalar(
                            out=q_sb[:], in0=m_sb[:],
                            scalar1=MAGIC, scalar2=None,
                            op0=mybir.AluOpType.subtract,
                        )
                        qv = q_sb[:].rearrange("p (r w) -> p r w", w=W)
                        nc.sync.dma_start(
                            out=lo_v[0:64, 1 + r0 : 1 + r0 + ROWS, 1 : 1 + W], in_=qv
                        )
                        nc.sync.dma_start(
                            out=lo_v[64:128, 1 + r0 : 1 + r0 + ROWS, 1 : 1 + W], in_=qv
                        )

                if t >= 1:
                    u = t - 1
                    x32 = x3pool.tile([128, SLICE // 2], F32, tag="x32")
                    nc.sync.dma_start(out=x32[:], in_=x32_d[:, u, :])
                    planes2 = [
                        pview((zerot if (u + kd) == 17 else qdup[u + kd])[:])
                        for kd in range(3)
                    ]
                    for p in range(NPAIR):
                        r0A, r0B = 2 * p * ROWS, (2 * p + 1) * ROWS
                        ps = ps2pool.tile([128, NT], F32, tag="ps2")
                        for i in range(27):
                            kd, kh, kw = i // 9, (i // 3) % 3, i % 3
                            first, last = i == 0, i == 26
                            wsl = w2hl[:, 64 * i : 64 * i + 64]
                            nc.tensor.matmul(
                                ps[0:64, :], wsl,
                                planes2[kd][:, r0A + kh : r0A + kh + ROWS, kw : kw + W],
                                start=first, stop=last, skip_group_check=True,
                            )
                            nc.tensor.matmul(
                                ps[64:128, :], wsl,
                                planes2[kd][:, r0B + kh : r0B + kh + ROWS, kw : kw + W],
                                start=first, stop=last, tile_position=(0, 64),
                                skip_group_check=True,
                            )
                        a2 = spool.tile([128, NT], F32, tag="a2")
                        nc.scalar.activation(
                            a2[:], ps[:], mybir.ActivationFunctionType.Identity,
                            bias=b2s2, scale=s2,
                        )
                        u2 = spool.tile([128, NT], F32, tag="u2")
                        nc.vector.tensor_scalar(
                            out=u2[:], in0=a2[:],
                            scalar1=MAGIC, scalar2=MAGIC - 127.0,
                            op0=mybir.AluOpType.add, op1=mybir.AluOpType.max,
                        )
                        y2 = spool.tile([128, NT], F32, tag="y2")
                        nc.vector.tensor_scalar(
                            out=y2[:], in0=u2[:],
                            scalar1=MAGIC + 127.0, scalar2=None,
                            op0=mybir.AluOpType.min,
                        )
                        ax = spool.tile([128, NT], F32, tag="ax")
                        nc.scalar.activation(
                            ax[:], x32[:, p * NT : (p + 1) * NT],
                            mybir.ActivationFunctionType.Identity,
                            bias=negM, scale=s2,
                        )
                        yx = spool.tile([128, NT], F32, tag="yx")
                        nc.vector.tensor_scalar(
                            out=yx[:], in0=ax[:],
                            scalar1=-MAGIC - 127.0, scalar2=-MAGIC + 127.0,
                            op0=mybir.AluOpType.max, op1=mybir.AluOpType.min,
                        )
                        z = spool.tile([128, NT], F32, tag="z")
                        nc.vector.tensor_tensor(
                            out=z[:], in0=y2[:], in1=yx[:], op=mybir.AluOpType.add
                        )
                        o_sb = spool.tile([128, NT], F32, tag="osb")
                        nc.scalar.activation(
                            o_sb[:], z[:], mybir.ActivationFunctionType.Relu,
                            bias=0.0, scale=inv_s2,
                        )
                        offA = u * SLICE + (2 * p) * NT
                        offB = u * SLICE + (2 * p + 1) * NT
                        nc.sync.dma_start(
                            out=out_d[:, offA : offA + NT], in_=o_sb[0:64, :]
                        )
                        nc.sync.dma_start(
                            out=out_d[:, offB : offB + NT], in_=o_sb[64:128, :]
                        )
    nc.compile()
    return nc


def _split_bf16(a):
    hi = a.astype(ml_dtypes.bfloat16)
    lo = (a - hi.astype(np.float32)).astype(ml_dtypes.bfloat16)
    return hi, lo


def _host_pack(x, w1, b1, w2, b2, exp1, exp2):
    scale1 = np.exp2(MANTISA_BIT - 1.0 - exp1).astype(np.float32)
    scale2 = np.exp2(MANTISA_BIT - 1.0 - exp2).astype(np.float32)

    w1t = np.transpose(w1, (2, 3, 4, 1, 0)).astype(np.float32)  # [kd,kh,kw,i,o]
    w1p9 = np.stack(
        [np.concatenate([w1t[0, kh, kw], w1t[1, kh, kw]], axis=0)
         for kh in range(3) for kw in range(3)]
    )  # [9,128,64]
    w1p9 = np.ascontiguousarray(np.transpose(w1p9, (1, 0, 2))).reshape(128, 9 * 64)
    w1c3 = np.stack(
        [np.concatenate([w1t[2, 0, kw], w1t[2, 1, kw]], axis=0) for kw in range(3)]
    )
    w1c3 = np.ascontiguousarray(np.transpose(w1c3, (1, 0, 2))).reshape(128, 3 * 64)
    w1k3 = np.stack([w1t[2, 2, kw] for kw in range(3)])
    w1k3 = np.ascontiguousarray(np.transpose(w1k3, (1, 0, 2))).reshape(64, 3 * 64)

    w2f = (w2 / scale1[None, :, None, None, None]).astype(np.float32)
    w2t = np.transpose(w2f, (2, 3, 4, 1, 0)).astype(np.float32)
    hi, lo = _split_bf16(w2t)
    w2hl = np.concatenate(
        [hi.reshape(27, 64, 64), lo.reshape(27, 64, 64)], axis=1
    )
    w2hl = np.ascontiguousarray(np.transpose(w2hl, (1, 0, 2))).reshape(128, 27 * 64)

    c64 = np.zeros((64, 8), dtype=np.float32)
    c64[:, 0] = scale1
    c64[:, 1] = b1 * scale1
    c64[:, 2] = scale2
    c64[:, 3] = b2 * scale2
    c64[:, 4] = 1.0 / scale2
    c64[:, 5] = -MAGIC
    coeff = np.concatenate([c64, c64], axis=0)

    shared = {"w1p9": w1p9, "w1c3": w1c3, "w1k3": w1k3, "w2hl": w2hl, "coeff": coeff}
    in_maps = []
    for n in range(N):
        xp = np.pad(x[n], ((0, 0), (1, 1), (1, 1), (1, 1))).astype(np.float32)
        m = dict(shared)
        m["xpad"] = np.ascontiguousarray(xp.reshape(C, TP, PLANE))
        xt = x[n].reshape(C, T, NTILES, NT)
        x32 = np.concatenate([xt[:, :, 0::2, :], xt[:, :, 1::2, :]], axis=0)
        m["x32"] = np.ascontiguousarray(
            x32.reshape(128, T, SLICE // 2).astype(np.float32)
        )
        in_maps.append(m)
    return in_maps


def kernel(x, w1, b1, w2, b2, exp1, exp2):
    global _COMPILED
    x = np.asarray(x, dtype=np.float32)
    w1 = np.asarray(w1, dtype=np.float32)
    b1 = np.asarray(b1, dtype=np.float32)
    w2 = np.asarray(w2, dtype=np.float32)
    b2 = np.asarray(b2, dtype=np.float32)
    exp1 = np.asarray(exp1, dtype=np.float32)
    exp2 = np.asarray(exp2, dtype=np.float32)
    if _COMPILED is None:
        _COMPILED = _build()
    in_maps = _host_pack(x, w1, b1, w2, b2, exp1, exp2)
    res = run_bass_kernel_spmd(_COMPILED, in_maps, core_ids=list(range(N)))
    out = np.stack([res.results[i]["out"].reshape(C, T, H, W) for i in range(N)])
    return out.astype(np.float32)

